# revision 10
# baseline (speedup 1.0000x reference)
"""Trainium2 Bass kernel for the Actor MLP (BatchNorm -> 3-layer MLP -> atan2).

Data-parallel across 8 NeuronCores: batch sharded 8192 rows/core, weights
replicated. BatchNorm batch stats via per-shard bn_stats + 8KB AllReduce.

v2 design (from baseline trace analysis, HW 2.20ms):
- The PE issue cadence was LDWEIGHTS-bound: fp32/f32r stationary loads take
  224ns (2 bus-cycles per 4-byte column) and cannot use the background
  weight buffer, so each LDW+MM pair costs 268ns even though the N=512
  stream is only 213ns. Fix: process batch tiles in PAIRS so each
  stationary weight tile serves two N=512 matmuls (cadence -> ~242ns/MM),
  which also halves the streamed-weight HBM traffic (403MB -> 202MB/core).
- x is pre-transposed on the host into [16, 128, 8, 512] tiles, removing
  all 1024 PE transposes (~131us) and the PSUM/ACT eviction pressure;
  bn_stats runs directly on the SBUF tiles with the exact same [128, 512]
  shapes/order as before (bit-identical stats -> identical rounding, since
  the 1.98e-2 rel err sits ~6 sign flips under the 2e-2 gate).
- The atan2 result stays in [action, batch] layout on device; the host
  transposes it back. L3 accumulates ty/tx in PSUM interleaved with L2
  (one m-slice behind) so h2 never fully materializes in SBUF.

Matmuls run in float32r (fp32 with the mantissa rounded to ~11 bits).
All matmul operand values, accumulation orders, and the bn_stats/bn_aggr
shapes are bit-identical to the validated baseline.
"""

import numpy as np

P = 128
B_CORE = 8192            # batch rows per core
BT = 512                 # batch tile (free dim of transposed activations)
NBT = B_CORE // BT       # 16
NPAIR = NBT // 2         # 8 pairs of batch tiles sharing weight loads
D_IN = 1024
K_IN = D_IN // P         # 8
D_H = 2048
K_H = D_H // P           # 16
D_ACT = 128
BN_EPS = 1e-5
N_CORES = 8
INV_PI = float(1.0 / np.pi)

_CACHE = {}

LAST_EXEC_NS = None
LAST_TRACE_DIR = None


def _build_nc():
    import concourse.mybir as mybir
    import concourse.tile as tile
    from concourse import bacc

    f32 = mybir.dt.float32
    f32r = mybir.dt.float32r
    AF = mybir.ActivationFunctionType
    ALU = mybir.AluOpType

    nc = bacc.Bacc()

    # x pre-transposed on host: [t, p, k, b] = x[t*512 + b, k*128 + p]
    x_ext = nc.declare_dram_parameter("x", [NBT, P, K_IN, BT], f32, isOutput=False)
    # weights pre-tiled on host: [m_slice, partition, k*128] so each m-slice
    # DMA reads one fully-contiguous block per partition
    w1t_ext = nc.declare_dram_parameter("w1t", [K_H, P, D_IN], f32r, isOutput=False)
    w2t_ext = nc.declare_dram_parameter("w2t", [K_H, P, D_H], f32r, isOutput=False)
    w3yt_ext = nc.declare_dram_parameter("w3yt", [D_H, D_ACT], f32r, isOutput=False)
    w3xt_ext = nc.declare_dram_parameter("w3xt", [D_H, D_ACT], f32r, isOutput=False)
    b1_ext = nc.declare_dram_parameter("b1r", [P, K_H], f32, isOutput=False)
    b2_ext = nc.declare_dram_parameter("b2r", [P, K_H], f32, isOutput=False)
    b3y_ext = nc.declare_dram_parameter("b3y", [P, 1], f32, isOutput=False)
    b3x_ext = nc.declare_dram_parameter("b3x", [P, 1], f32, isOutput=False)
    bnw_ext = nc.declare_dram_parameter("bnw", [P, K_IN], f32, isOutput=False)
    bnb_ext = nc.declare_dram_parameter("bnb", [P, K_IN], f32, isOutput=False)
    # output in transposed [action, batch] layout; host transposes back
    out_ext = nc.declare_dram_parameter("out", [D_ACT, B_CORE], f32, isOutput=True)

    with tile.TileContext(nc) as tc:
        with (
            tc.tile_pool(name="singles", bufs=1) as singles,
            tc.tile_pool(name="small", bufs=1) as small,
            tc.tile_pool(name="raw", bufs=8) as raw_pool,
            tc.tile_pool(name="xn", bufs=1) as xt_pool,
            tc.tile_pool(name="w1m", bufs=3) as w1_pool,
            tc.tile_pool(name="w2m", bufs=2) as w2_pool,
            tc.tile_pool(name="hid", bufs=1) as hid_pool,
            tc.tile_pool(name="h2s", bufs=4) as h2s_pool,
            tc.tile_pool(name="epi", bufs=2) as epi_pool,
            tc.tile_pool(name="mmpsum", bufs=4, space="PSUM") as mm_pool,
            tc.tile_pool(name="l3psum", bufs=1, space="PSUM") as l3_pool,
            tc.tile_pool(name="dram", bufs=1, space="DRAM") as dram_pool,
        ):
            # ---- constants ----
            bnws = singles.tile([P, K_IN], f32)
            nc.sync.dma_start(out=bnws, in_=bnw_ext[:])
            bnbs = singles.tile([P, K_IN], f32)
            nc.sync.dma_start(out=bnbs, in_=bnb_ext[:])

            # ---- pass 1: batch stats straight off the host-transposed tiles
            # (same [128, 512] per-(k,t) bn_stats and per-k bn_aggr as the
            # validated baseline -> bit-identical mean/var) ----
            stats = small.tile([P, K_IN, NBT, 6], f32)
            mv = small.tile([P, K_IN, 2], f32)
            # pk = [mean, E[x^2]] / N_CORES, packed for the AllReduce
            pk = small.tile([P, K_IN, 2], f32)
            KH4 = K_IN // 4
            for t in range(NBT):
                for h in range(4):
                    raw = raw_pool.tile([P, KH4, BT], f32, tag="raw", name="rawh")
                    nc.sync.dma_start(out=raw, in_=x_ext[t, :, h * KH4 : (h + 1) * KH4])
                    for kk in range(KH4):
                        k = h * KH4 + kk
                        nc.vector.bn_stats(out=stats[:, k, t, :], in_=raw[:, kk, :])
                        if t == NBT - 1:
                            # aggregate per k as soon as its last bn_stats lands
                            nc.vector.bn_aggr(out=mv[:, k, :], in_=stats[:, k, :, :])

            # pass-2 constants, loaded behind the stats pass
            w3ys = singles.tile([P, K_H, D_ACT], f32r)
            nc.sync.dma_start(out=w3ys, in_=w3yt_ext[:].rearrange("(k p) a -> p k a", p=P))
            w3xs = singles.tile([P, K_H, D_ACT], f32r)
            nc.sync.dma_start(out=w3xs, in_=w3xt_ext[:].rearrange("(k p) a -> p k a", p=P))
            b1s = singles.tile([P, K_H], f32)
            nc.sync.dma_start(out=b1s, in_=b1_ext[:])
            b2s = singles.tile([P, K_H], f32)
            nc.sync.dma_start(out=b2s, in_=b2_ext[:])
            b3ys = singles.tile([P, 1], f32)
            nc.sync.dma_start(out=b3ys, in_=b3y_ext[:])
            b3xs = singles.tile([P, 1], f32)
            nc.sync.dma_start(out=b3xs, in_=b3x_ext[:])

            # pack [mean, E[x^2]]/N_CORES for the AllReduce
            nc.vector.tensor_scalar_mul(pk[:, :, 0], mv[:, :, 0], 1.0 / N_CORES)
            nc.vector.tensor_mul(pk[:, :, 1], mv[:, :, 0], mv[:, :, 0])
            nc.vector.tensor_add(pk[:, :, 1], pk[:, :, 1], mv[:, :, 1])
            nc.vector.tensor_scalar_mul(pk[:, :, 1], pk[:, :, 1], 1.0 / N_CORES)

            cc_in = dram_pool.tile([P, K_IN, 2], f32)
            cc_out = dram_pool.tile([P, K_IN, 2], f32)
            # gpsimd's queue, so this 8KB doesn't sit behind weight MBs
            nc.scalar.dma_start(out=cc_in, in_=pk)
            nc.gpsimd.collective_compute(
                "AllReduce",
                ALU.add,
                replica_groups=[list(range(N_CORES))],
                ins=[cc_in.opt()],
                outs=[cc_out.opt()],
            )
            g = small.tile([P, K_IN, 2], f32)
            nc.scalar.dma_start(out=g, in_=cc_out)

            # global mean / var -> per-feature scale & shift
            gm = g[:, :, 0]
            var = small.tile([P, K_IN], f32)
            nc.vector.tensor_copy(out=var, in_=g[:, :, 1])
            gm2 = small.tile([P, K_IN], f32)
            nc.vector.tensor_mul(gm2, gm, gm)
            nc.vector.tensor_sub(var, var, gm2)
            eps_t = small.tile([P, 1], f32)
            nc.vector.memset(eps_t, BN_EPS)
            sq = small.tile([P, K_IN], f32)
            nc.scalar.activation(out=sq, in_=var, func=AF.Sqrt, bias=eps_t, scale=1.0)
            rstd = small.tile([P, K_IN], f32)
            nc.vector.reciprocal(out=rstd, in_=sq)
            scale = small.tile([P, K_IN], f32)
            nc.vector.tensor_mul(scale, bnws, rstd)
            shift = small.tile([P, K_IN], f32)
            nc.vector.tensor_mul(shift, gm, scale)
            nc.vector.tensor_sub(shift, bnbs, shift)

            # ---- pass 2: normalize + 3-layer MLP + atan2, paired batch tiles
            # so each stationary weight tile feeds two N=512 matmuls ----
            for pr in range(NPAIR):
                tiles = (2 * pr, 2 * pr + 1)
                xts = []
                for t in tiles:
                    xt = xt_pool.tile([P, K_IN, BT], f32r, tag=f"xt{t % 2}")
                    for h in range(4):
                        raw = raw_pool.tile([P, KH4, BT], f32, tag="raw", name="rawh")
                        nc.sync.dma_start(
                            out=raw, in_=x_ext[t, :, h * KH4 : (h + 1) * KH4]
                        )
                        for kk in range(KH4):
                            k = h * KH4 + kk
                            nc.scalar.activation(
                                out=xt[:, k, :],
                                in_=raw[:, kk, :],
                                func=AF.Identity,
                                bias=shift[:, k : k + 1],
                                scale=scale[:, k : k + 1],
                            )
                    xts.append(xt)

                # layer 1: h1T = relu(W1 @ xnormT + b1), W1 streamed by m-slice
                h1s = [
                    hid_pool.tile([P, K_H, BT], f32r, tag=f"h1{i}", name=f"h1{i}")
                    for i in range(2)
                ]
                for m in range(K_H):
                    w1m = w1_pool.tile([P, K_IN, P], f32r, tag="w1m")
                    nc.sync.dma_start(
                        out=w1m,
                        in_=w1t_ext[m].rearrange("p (k c) -> p k c", k=K_IN),
                    )
                    accs = [
                        mm_pool.tile([P, BT], f32, tag="mm", name=f"acc{i}")
                        for i in range(2)
                    ]
                    for k in range(K_IN):
                        for i in range(2):
                            nc.tensor.matmul(
                                accs[i],
                                w1m[:, k, :],
                                xts[i][:, k, :],
                                start=(k == 0),
                                stop=(k == K_IN - 1),
                            )
                    for i in range(2):
                        nc.scalar.activation(
                            out=h1s[i][:, m, :],
                            in_=accs[i],
                            func=AF.Relu,
                            bias=b1s[:, m : m + 1],
                            scale=1.0,
                        )

                # layer 2 + layer 3 interleaved: as each h2 m-slice is evicted,
                # its ty/tx contributions accumulate one m-iteration later so
                # the PE never waits on the ACT eviction.
                tys = [
                    l3_pool.tile([P, BT], f32, tag=f"ty{i}", name=f"ty{i}")
                    for i in range(2)
                ]
                txs = [
                    l3_pool.tile([P, BT], f32, tag=f"tx{i}", name=f"tx{i}")
                    for i in range(2)
                ]
                h2hist = {}
                for m in range(K_H):
                    w2m = w2_pool.tile([P, K_H, P], f32r, tag="w2m")
                    nc.sync.dma_start(
                        out=w2m,
                        in_=w2t_ext[m].rearrange("p (k c) -> p k c", k=K_H),
                    )
                    accs = [
                        mm_pool.tile([P, BT], f32, tag="mm", name=f"acc{i}")
                        for i in range(2)
                    ]
                    for k in range(K_H):
                        for i in range(2):
                            nc.tensor.matmul(
                                accs[i],
                                w2m[:, k, :],
                                h1s[i][:, k, :],
                                start=(k == 0),
                                stop=(k == K_H - 1),
                            )
                    h2m = [
                        h2s_pool.tile([P, BT], f32r, tag="h2s", name=f"h2s{i}")
                        for i in range(2)
                    ]
                    for i in range(2):
                        nc.scalar.activation(
                            out=h2m[i],
                            in_=accs[i],
                            func=AF.Relu,
                            bias=b2s[:, m : m + 1],
                            scale=1.0,
                        )
                    h2hist[m] = h2m
                    if m >= 1:
                        _emit_l3(nc, m - 1, h2hist.pop(m - 1), w3ys, w3xs, tys, txs, K_H)
                _emit_l3(nc, K_H - 1, h2hist.pop(K_H - 1), w3ys, w3xs, tys, txs, K_H)

                # tanh eviction + atan2 epilogue per tile (off the PE)
                for i, t in enumerate(tiles):
                    ty = epi_pool.tile([P, BT], f32, tag="ty")
                    tx = epi_pool.tile([P, BT], f32, tag="tx")
                    nc.scalar.activation(out=ty, in_=tys[i], func=AF.Tanh, bias=b3ys, scale=1.0)
                    nc.scalar.activation(out=tx, in_=txs[i], func=AF.Tanh, bias=b3xs, scale=1.0)

                    # atan2(ty, tx)/pi = Arctan(ty/tx)/pi + sign(ty)*(1-sign(tx))/2
                    rx = epi_pool.tile([P, BT], f32, tag="rx")
                    nc.vector.reciprocal(out=rx, in_=tx)
                    nc.vector.tensor_mul(rx, ty, rx)            # q = ty/tx
                    sy = epi_pool.tile([P, BT], f32, tag="sy")
                    nc.scalar.activation(out=sy, in_=ty, func=AF.Sign)
                    sx = epi_pool.tile([P, BT], f32, tag="sx")
                    nc.scalar.activation(out=sx, in_=tx, func=AF.Sign)
                    nc.scalar.activation(out=tx, in_=rx, func=AF.Arctan)  # a (tx dead)
                    nc.vector.tensor_mul(sx, sy, sx)            # sy*sx
                    nc.vector.tensor_sub(sy, sy, sx)            # d = sy*(1-sx)
                    nc.vector.tensor_scalar(
                        out=rx, in0=tx, scalar1=INV_PI, scalar2=None, op0=ALU.mult
                    )
                    nc.vector.tensor_scalar(
                        out=sy, in0=sy, scalar1=0.5, scalar2=None, op0=ALU.mult
                    )
                    nc.vector.tensor_add(rx, rx, sy)            # resT
                    nc.sync.dma_start(
                        out=out_ext[:, t * BT : (t + 1) * BT], in_=rx
                    )

    return nc


def _emit_l3(nc, m, h2m, w3ys, w3xs, tys, txs, K_H):
    """ty/tx += w3{y,x}[m] @ h2[m] for both tiles of the pair; each stationary
    serves two matmuls. PSUM accumulation order over m matches the baseline."""
    for ws, dsts in ((w3ys, tys), (w3xs, txs)):
        for i in range(2):
            nc.tensor.matmul(
                dsts[i],
                ws[:, m, :],
                h2m[i],
                start=(m == 0),
                stop=(m == K_H - 1),
            )


def _round_f32r(a):
    """Round-to-nearest to f32r granularity (low 12 mantissa bits zeroed)."""
    a = np.ascontiguousarray(np.asarray(a, np.float32))
    b = a.view(np.uint32)
    b = ((b + 0x800) & np.uint32(0xFFFFF000)).astype(np.uint32)
    return b.view(np.float32)


def _tile_w(w, k_tiles):
    """[M, K] row-major -> [M/128, 128p, K] where [m, p, k*128+c] = w[m*128+c, k*128+p]."""
    m_tiles = w.shape[0] // P
    t = w.reshape(m_tiles, P, k_tiles, P).transpose(0, 3, 2, 1)
    return np.ascontiguousarray(t.reshape(m_tiles, P, k_tiles * P))


def _host_prep(states, bn_weight, bn_bias, w1, b1, w2, b2, w3, b3):
    w1t = _round_f32r(_tile_w(np.asarray(w1, np.float32), K_IN))
    w2t = _round_f32r(_tile_w(np.asarray(w2, np.float32), K_H))
    w3 = np.asarray(w3, np.float32)
    w3yt = _round_f32r(w3[0::2].T)   # [D_H, D_ACT]
    w3xt = _round_f32r(w3[1::2].T)
    b1r = np.ascontiguousarray(np.asarray(b1, np.float32).reshape(K_H, P).T)
    b2r = np.ascontiguousarray(np.asarray(b2, np.float32).reshape(K_H, P).T)
    b3 = np.asarray(b3, np.float32)
    b3y = np.ascontiguousarray(b3[0::2].reshape(P, 1))
    b3x = np.ascontiguousarray(b3[1::2].reshape(P, 1))
    bnw = np.ascontiguousarray(np.asarray(bn_weight, np.float32).reshape(K_IN, P).T)
    bnb = np.ascontiguousarray(np.asarray(bn_bias, np.float32).reshape(K_IN, P).T)
    shared = {
        "w1t": w1t, "w2t": w2t, "w3yt": w3yt, "w3xt": w3xt,
        "b1r": b1r, "b2r": b2r, "b3y": b3y, "b3x": b3x,
        "bnw": bnw, "bnb": bnb,
    }
    states = np.asarray(states, np.float32)
    in_maps = []
    for c in range(N_CORES):
        m = dict(shared)
        shard = states[c * B_CORE : (c + 1) * B_CORE]
        # [t, p, k, b] = x[t*512 + b, k*128 + p]
        m["x"] = np.ascontiguousarray(
            shard.reshape(NBT, BT, K_IN, P).transpose(0, 3, 2, 1)
        )
        in_maps.append(m)
    return in_maps


def _get_ntff_hook():
    """Best-effort NTFF profiling hook (axon images without antenv.axon_hooks)."""
    try:
        from antenv.axon_hooks import get_axon_ntff_profile_hook

        return get_axon_ntff_profile_hook()
    except ImportError:
        pass
    try:
        from trn_agent_boot.trn_boot import _ntff_profile_via_ctypes

        return _ntff_profile_via_ctypes("/opt/axon/libaxon_pjrt.so")
    except Exception:
        return None


def _run(nc, in_maps, profile=True):
    """Run the SPMD kernel via PJRT; return (per-core results, exec_time_ns)."""
    import glob
    import os
    import tempfile

    from concourse import bass2jax

    hook = _get_ntff_hook() if profile else None
    if hook is None:
        return bass2jax.run_bass_via_pjrt(nc, in_maps, n_cores=N_CORES), None, None

    tmpdir = tempfile.mkdtemp(prefix="bass_ntff_")
    try:
        with hook(tmpdir, [0]):
            results = bass2jax.run_bass_via_pjrt(nc, in_maps, n_cores=N_CORES)
    except Exception as e:
        print(f"[kernel] NTFF hook failed ({type(e).__name__}: {e}); plain run")
        return bass2jax.run_bass_via_pjrt(nc, in_maps, n_cores=N_CORES), None, None

    exec_ns = None
    try:
        if glob.glob(os.path.join(tmpdir, "*_body*.ntff")):
            import gauge.profiler
            from concourse._compat import FishPath

            profile_obj = gauge.profiler.Profile(
                profile_path=FishPath(tmpdir),
                kernel_dev_mode=True,
                profile_on_exit=False,
                bass_kernel=nc.m,
                offline_processing=True,
                fname="*_body*",
            )
            prs = profile_obj.to_perfetto(model_index=(0,))
            if prs:
                exec_ns = max(p.exec_time_ns for p in prs if p.exec_time_ns)
    except Exception as e:
        print(f"[kernel] NTFF parse failed ({type(e).__name__}: {e})")
    return results, exec_ns, tmpdir


def kernel(**inputs):
    global LAST_EXEC_NS, LAST_TRACE_DIR
    if "nc" not in _CACHE:
        nc = _build_nc()
        if not nc.is_finalized():
            nc.finalize()
        _CACHE["nc"] = nc
    nc = _CACHE["nc"]

    in_maps = _host_prep(**inputs)
    results, exec_ns, trace_dir = _run(nc, in_maps)
    LAST_EXEC_NS = exec_ns
    LAST_TRACE_DIR = trace_dir
    out = np.concatenate(
        [np.ascontiguousarray(results[c]["out"].T) for c in range(N_CORES)], axis=0
    )
    return out.astype(np.float32)


# revision 11
# speedup vs baseline: 1.0323x; 1.0323x over previous
"""Trainium2 Bass kernel for the Actor MLP (BatchNorm -> 3-layer MLP -> atan2).

Data-parallel across 8 NeuronCores: batch sharded 8192 rows/core, weights
replicated. BatchNorm batch stats via per-shard bn_stats + 8KB AllReduce.

v2 design (from baseline trace analysis, HW 2.20ms):
- The PE issue cadence was LDWEIGHTS-bound: fp32/f32r stationary loads take
  224ns (2 bus-cycles per 4-byte column) and cannot use the background
  weight buffer, so each LDW+MM pair costs 268ns even though the N=512
  stream is only 213ns. Fix: process batch tiles in PAIRS so each
  stationary weight tile serves two N=512 matmuls (cadence -> ~242ns/MM),
  which also halves the streamed-weight HBM traffic (403MB -> 202MB/core).
- x is pre-transposed on the host into [16, 128, 8, 512] tiles, removing
  all 1024 PE transposes (~131us) and the PSUM/ACT eviction pressure;
  bn_stats runs directly on the SBUF tiles with the exact same [128, 512]
  shapes/order as before (bit-identical stats -> identical rounding, since
  the 1.98e-2 rel err sits ~6 sign flips under the 2e-2 gate).
- The atan2 result stays in [action, batch] layout on device; the host
  transposes it back. L3 accumulates ty/tx in PSUM interleaved with L2
  (one m-slice behind) so h2 never fully materializes in SBUF.

Matmuls run in float32r (fp32 with the mantissa rounded to ~11 bits).
All matmul operand values, accumulation orders, and the bn_stats/bn_aggr
shapes are bit-identical to the validated baseline.
"""

import numpy as np

P = 128
B_CORE = 8192            # batch rows per core
BT = 512                 # batch tile (free dim of transposed activations)
NBT = B_CORE // BT       # 16
NPAIR = NBT // 2         # 8 pairs of batch tiles sharing weight loads
D_IN = 1024
K_IN = D_IN // P         # 8
D_H = 2048
K_H = D_H // P           # 16
D_ACT = 128
BN_EPS = 1e-5
N_CORES = 8
INV_PI = float(1.0 / np.pi)

_CACHE = {}

LAST_EXEC_NS = None
LAST_TRACE_DIR = None


def _build_nc():
    import concourse.mybir as mybir
    import concourse.tile as tile
    from concourse import bacc

    f32 = mybir.dt.float32
    f32r = mybir.dt.float32r
    AF = mybir.ActivationFunctionType
    ALU = mybir.AluOpType

    nc = bacc.Bacc()

    # x pre-transposed on host: [t, p, k, b] = x[t*512 + b, k*128 + p]
    x_ext = nc.declare_dram_parameter("x", [NBT, P, K_IN, BT], f32, isOutput=False)
    # weights pre-tiled on host: [m_slice, partition, k*128] so each m-slice
    # DMA reads one fully-contiguous block per partition
    w1t_ext = nc.declare_dram_parameter("w1t", [K_H, P, D_IN], f32r, isOutput=False)
    w2t_ext = nc.declare_dram_parameter("w2t", [K_H, P, D_H], f32r, isOutput=False)
    w3yt_ext = nc.declare_dram_parameter("w3yt", [D_H, D_ACT], f32r, isOutput=False)
    w3xt_ext = nc.declare_dram_parameter("w3xt", [D_H, D_ACT], f32r, isOutput=False)
    b1_ext = nc.declare_dram_parameter("b1r", [P, K_H], f32, isOutput=False)
    b2_ext = nc.declare_dram_parameter("b2r", [P, K_H], f32, isOutput=False)
    b3y_ext = nc.declare_dram_parameter("b3y", [P, 1], f32, isOutput=False)
    b3x_ext = nc.declare_dram_parameter("b3x", [P, 1], f32, isOutput=False)
    bnw_ext = nc.declare_dram_parameter("bnw", [P, K_IN], f32, isOutput=False)
    bnb_ext = nc.declare_dram_parameter("bnb", [P, K_IN], f32, isOutput=False)
    # output in transposed [action, batch] layout; host transposes back
    out_ext = nc.declare_dram_parameter("out", [D_ACT, B_CORE], f32, isOutput=True)

    with tile.TileContext(nc) as tc:
        with (
            tc.tile_pool(name="singles", bufs=1) as singles,
            tc.tile_pool(name="small", bufs=1) as small,
            tc.tile_pool(name="raw", bufs=4) as raw_pool,
            tc.tile_pool(name="xn", bufs=1) as xt_pool,
            tc.tile_pool(name="w1m", bufs=3) as w1_pool,
            tc.tile_pool(name="w2m", bufs=2) as w2_pool,
            tc.tile_pool(name="hid", bufs=1) as hid_pool,
            tc.tile_pool(name="h2s", bufs=4) as h2s_pool,
            tc.tile_pool(name="epi", bufs=1) as epi_pool,
            tc.tile_pool(name="mmpsum", bufs=4, space="PSUM") as mm_pool,
            tc.tile_pool(name="l3psum", bufs=1, space="PSUM") as l3_pool,
            tc.tile_pool(name="dram", bufs=1, space="DRAM") as dram_pool,
        ):
            # ---- constants ----
            bnws = singles.tile([P, K_IN], f32)
            nc.sync.dma_start(out=bnws, in_=bnw_ext[:])
            bnbs = singles.tile([P, K_IN], f32)
            nc.sync.dma_start(out=bnbs, in_=bnb_ext[:])

            # ---- pass 1: batch stats straight off the host-transposed tiles
            # (same [128, 512] per-(k,t) bn_stats and per-k bn_aggr as the
            # validated baseline -> bit-identical mean/var) ----
            stats = small.tile([P, K_IN, NBT, 6], f32)
            mv = small.tile([P, K_IN, 2], f32)
            # pk = [mean, E[x^2]] / N_CORES, packed for the AllReduce
            pk = small.tile([P, K_IN, 2], f32)
            KH2 = K_IN // 2
            for t in range(NBT):
                for h in range(2):
                    raw = raw_pool.tile([P, KH2, BT], f32, tag="raw", name="rawh")
                    nc.sync.dma_start(out=raw, in_=x_ext[t, :, h * KH2 : (h + 1) * KH2])
                    for kk in range(KH2):
                        k = h * KH2 + kk
                        nc.vector.bn_stats(out=stats[:, k, t, :], in_=raw[:, kk, :])
                        if t == NBT - 1:
                            # aggregate per k as soon as its last bn_stats lands
                            nc.vector.bn_aggr(out=mv[:, k, :], in_=stats[:, k, :, :])

            # pass-2 constants, loaded behind the stats pass
            w3ys = singles.tile([P, K_H, D_ACT], f32r)
            nc.sync.dma_start(out=w3ys, in_=w3yt_ext[:].rearrange("(k p) a -> p k a", p=P))
            w3xs = singles.tile([P, K_H, D_ACT], f32r)
            nc.sync.dma_start(out=w3xs, in_=w3xt_ext[:].rearrange("(k p) a -> p k a", p=P))
            b1s = singles.tile([P, K_H], f32)
            nc.sync.dma_start(out=b1s, in_=b1_ext[:])
            b2s = singles.tile([P, K_H], f32)
            nc.sync.dma_start(out=b2s, in_=b2_ext[:])
            b3ys = singles.tile([P, 1], f32)
            nc.sync.dma_start(out=b3ys, in_=b3y_ext[:])
            b3xs = singles.tile([P, 1], f32)
            nc.sync.dma_start(out=b3xs, in_=b3x_ext[:])

            # pack [mean, E[x^2]]/N_CORES for the AllReduce
            nc.vector.tensor_scalar_mul(pk[:, :, 0], mv[:, :, 0], 1.0 / N_CORES)
            nc.vector.tensor_mul(pk[:, :, 1], mv[:, :, 0], mv[:, :, 0])
            nc.vector.tensor_add(pk[:, :, 1], pk[:, :, 1], mv[:, :, 1])
            nc.vector.tensor_scalar_mul(pk[:, :, 1], pk[:, :, 1], 1.0 / N_CORES)

            cc_in = dram_pool.tile([P, K_IN, 2], f32)
            cc_out = dram_pool.tile([P, K_IN, 2], f32)
            # gpsimd's queue, so this 8KB doesn't sit behind weight MBs
            nc.gpsimd.dma_start(out=cc_in, in_=pk)
            nc.gpsimd.collective_compute(
                "AllReduce",
                ALU.add,
                replica_groups=[list(range(N_CORES))],
                ins=[cc_in.opt()],
                outs=[cc_out.opt()],
            )
            g = small.tile([P, K_IN, 2], f32)
            nc.gpsimd.dma_start(out=g, in_=cc_out)

            # global mean / var -> per-feature scale & shift
            gm = g[:, :, 0]
            var = small.tile([P, K_IN], f32)
            nc.vector.tensor_copy(out=var, in_=g[:, :, 1])
            gm2 = small.tile([P, K_IN], f32)
            nc.vector.tensor_mul(gm2, gm, gm)
            nc.vector.tensor_sub(var, var, gm2)
            eps_t = small.tile([P, 1], f32)
            nc.vector.memset(eps_t, BN_EPS)
            sq = small.tile([P, K_IN], f32)
            nc.scalar.activation(out=sq, in_=var, func=AF.Sqrt, bias=eps_t, scale=1.0)
            rstd = small.tile([P, K_IN], f32)
            nc.vector.reciprocal(out=rstd, in_=sq)
            scale = small.tile([P, K_IN], f32)
            nc.vector.tensor_mul(scale, bnws, rstd)
            shift = small.tile([P, K_IN], f32)
            nc.vector.tensor_mul(shift, gm, scale)
            nc.vector.tensor_sub(shift, bnbs, shift)

            # ---- pass 2: normalize + 3-layer MLP + atan2, paired batch tiles
            # so each stationary weight tile feeds two N=512 matmuls ----
            for pr in range(NPAIR):
                tiles = (2 * pr, 2 * pr + 1)
                xts = []
                for t in tiles:
                    xt = xt_pool.tile([P, K_IN, BT], f32r, tag=f"xt{t % 2}")
                    for h in range(2):
                        raw = raw_pool.tile([P, KH2, BT], f32, tag="raw", name="rawh")
                        nc.sync.dma_start(
                            out=raw, in_=x_ext[t, :, h * KH2 : (h + 1) * KH2]
                        )
                        for kk in range(KH2):
                            k = h * KH2 + kk
                            nc.scalar.activation(
                                out=xt[:, k, :],
                                in_=raw[:, kk, :],
                                func=AF.Identity,
                                bias=shift[:, k : k + 1],
                                scale=scale[:, k : k + 1],
                            )
                    xts.append(xt)

                # layer 1: h1T = relu(W1 @ xnormT + b1), W1 streamed by m-slice
                h1s = [
                    hid_pool.tile([P, K_H, BT], f32r, tag=f"h1{i}", name=f"h1{i}")
                    for i in range(2)
                ]
                for m in range(K_H):
                    w1m = w1_pool.tile([P, K_IN, P], f32r, tag="w1m")
                    nc.sync.dma_start(
                        out=w1m,
                        in_=w1t_ext[m].rearrange("p (k c) -> p k c", k=K_IN),
                    )
                    accs = [
                        mm_pool.tile([P, BT], f32, tag="mm", name=f"acc{i}")
                        for i in range(2)
                    ]
                    for k in range(K_IN):
                        for i in range(2):
                            nc.tensor.matmul(
                                accs[i],
                                w1m[:, k, :],
                                xts[i][:, k, :],
                                start=(k == 0),
                                stop=(k == K_IN - 1),
                            )
                    for i in range(2):
                        nc.scalar.activation(
                            out=h1s[i][:, m, :],
                            in_=accs[i],
                            func=AF.Relu,
                            bias=b1s[:, m : m + 1],
                            scale=1.0,
                        )

                # layer 2 + layer 3 interleaved: as each h2 m-slice is evicted,
                # its ty/tx contributions accumulate one m-iteration later so
                # the PE never waits on the ACT eviction.
                tys = [
                    l3_pool.tile([P, BT], f32, tag=f"ty{i}", name=f"ty{i}")
                    for i in range(2)
                ]
                txs = [
                    l3_pool.tile([P, BT], f32, tag=f"tx{i}", name=f"tx{i}")
                    for i in range(2)
                ]
                h2hist = {}
                for m in range(K_H):
                    w2m = w2_pool.tile([P, K_H, P], f32r, tag="w2m")
                    nc.sync.dma_start(
                        out=w2m,
                        in_=w2t_ext[m].rearrange("p (k c) -> p k c", k=K_H),
                    )
                    accs = [
                        mm_pool.tile([P, BT], f32, tag="mm", name=f"acc{i}")
                        for i in range(2)
                    ]
                    for k in range(K_H):
                        for i in range(2):
                            nc.tensor.matmul(
                                accs[i],
                                w2m[:, k, :],
                                h1s[i][:, k, :],
                                start=(k == 0),
                                stop=(k == K_H - 1),
                            )
                    h2m = [
                        h2s_pool.tile([P, BT], f32r, tag="h2s", name=f"h2s{i}")
                        for i in range(2)
                    ]
                    for i in range(2):
                        nc.scalar.activation(
                            out=h2m[i],
                            in_=accs[i],
                            func=AF.Relu,
                            bias=b2s[:, m : m + 1],
                            scale=1.0,
                        )
                    h2hist[m] = h2m
                    if m >= 1:
                        _emit_l3(nc, m - 1, h2hist.pop(m - 1), w3ys, w3xs, tys, txs, K_H)
                _emit_l3(nc, K_H - 1, h2hist.pop(K_H - 1), w3ys, w3xs, tys, txs, K_H)

                # tanh eviction + atan2 epilogue per tile (off the PE)
                for i, t in enumerate(tiles):
                    ty = epi_pool.tile([P, BT], f32, tag="ty")
                    tx = epi_pool.tile([P, BT], f32, tag="tx")
                    nc.scalar.activation(out=ty, in_=tys[i], func=AF.Tanh, bias=b3ys, scale=1.0)
                    nc.scalar.activation(out=tx, in_=txs[i], func=AF.Tanh, bias=b3xs, scale=1.0)

                    # atan2(ty, tx)/pi = Arctan(ty/tx)/pi + sign(ty)*(1-sign(tx))/2
                    rx = epi_pool.tile([P, BT], f32, tag="rx")
                    nc.vector.reciprocal(out=rx, in_=tx)
                    nc.vector.tensor_mul(rx, ty, rx)            # q = ty/tx
                    sy = epi_pool.tile([P, BT], f32, tag="sy")
                    nc.scalar.activation(out=sy, in_=ty, func=AF.Sign)
                    sx = epi_pool.tile([P, BT], f32, tag="sx")
                    nc.scalar.activation(out=sx, in_=tx, func=AF.Sign)
                    nc.scalar.activation(out=tx, in_=rx, func=AF.Arctan)  # a (tx dead)
                    nc.vector.tensor_mul(sx, sy, sx)            # sy*sx
                    nc.vector.tensor_sub(sy, sy, sx)            # d = sy*(1-sx)
                    nc.vector.tensor_scalar(
                        out=rx, in0=tx, scalar1=INV_PI, scalar2=None, op0=ALU.mult
                    )
                    nc.vector.tensor_scalar(
                        out=sy, in0=sy, scalar1=0.5, scalar2=None, op0=ALU.mult
                    )
                    nc.vector.tensor_add(rx, rx, sy)            # resT
                    nc.sync.dma_start(
                        out=out_ext[:, t * BT : (t + 1) * BT], in_=rx
                    )

    return nc


def _emit_l3(nc, m, h2m, w3ys, w3xs, tys, txs, K_H):
    """ty/tx += w3{y,x}[m] @ h2[m] for both tiles of the pair; each stationary
    serves two matmuls. PSUM accumulation order over m matches the baseline."""
    for ws, dsts in ((w3ys, tys), (w3xs, txs)):
        for i in range(2):
            nc.tensor.matmul(
                dsts[i],
                ws[:, m, :],
                h2m[i],
                start=(m == 0),
                stop=(m == K_H - 1),
            )


def _round_f32r(a):
    """Round-to-nearest to f32r granularity (low 12 mantissa bits zeroed)."""
    a = np.ascontiguousarray(np.asarray(a, np.float32))
    b = a.view(np.uint32)
    b = ((b + 0x800) & np.uint32(0xFFFFF000)).astype(np.uint32)
    return b.view(np.float32)


def _tile_w(w, k_tiles):
    """[M, K] row-major -> [M/128, 128p, K] where [m, p, k*128+c] = w[m*128+c, k*128+p]."""
    m_tiles = w.shape[0] // P
    t = w.reshape(m_tiles, P, k_tiles, P).transpose(0, 3, 2, 1)
    return np.ascontiguousarray(t.reshape(m_tiles, P, k_tiles * P))


def _host_prep(states, bn_weight, bn_bias, w1, b1, w2, b2, w3, b3):
    w1t = _round_f32r(_tile_w(np.asarray(w1, np.float32), K_IN))
    w2t = _round_f32r(_tile_w(np.asarray(w2, np.float32), K_H))
    w3 = np.asarray(w3, np.float32)
    w3yt = _round_f32r(w3[0::2].T)   # [D_H, D_ACT]
    w3xt = _round_f32r(w3[1::2].T)
    b1r = np.ascontiguousarray(np.asarray(b1, np.float32).reshape(K_H, P).T)
    b2r = np.ascontiguousarray(np.asarray(b2, np.float32).reshape(K_H, P).T)
    b3 = np.asarray(b3, np.float32)
    b3y = np.ascontiguousarray(b3[0::2].reshape(P, 1))
    b3x = np.ascontiguousarray(b3[1::2].reshape(P, 1))
    bnw = np.ascontiguousarray(np.asarray(bn_weight, np.float32).reshape(K_IN, P).T)
    bnb = np.ascontiguousarray(np.asarray(bn_bias, np.float32).reshape(K_IN, P).T)
    shared = {
        "w1t": w1t, "w2t": w2t, "w3yt": w3yt, "w3xt": w3xt,
        "b1r": b1r, "b2r": b2r, "b3y": b3y, "b3x": b3x,
        "bnw": bnw, "bnb": bnb,
    }
    states = np.asarray(states, np.float32)
    in_maps = []
    for c in range(N_CORES):
        m = dict(shared)
        shard = states[c * B_CORE : (c + 1) * B_CORE]
        # [t, p, k, b] = x[t*512 + b, k*128 + p]
        m["x"] = np.ascontiguousarray(
            shard.reshape(NBT, BT, K_IN, P).transpose(0, 3, 2, 1)
        )
        in_maps.append(m)
    return in_maps


def _get_ntff_hook():
    """Best-effort NTFF profiling hook (axon images without antenv.axon_hooks)."""
    try:
        from antenv.axon_hooks import get_axon_ntff_profile_hook

        return get_axon_ntff_profile_hook()
    except ImportError:
        pass
    try:
        from trn_agent_boot.trn_boot import _ntff_profile_via_ctypes

        return _ntff_profile_via_ctypes("/opt/axon/libaxon_pjrt.so")
    except Exception:
        return None


def _run(nc, in_maps, profile=True):
    """Run the SPMD kernel via PJRT; return (per-core results, exec_time_ns)."""
    import glob
    import os
    import tempfile

    from concourse import bass2jax

    hook = _get_ntff_hook() if profile else None
    if hook is None:
        return bass2jax.run_bass_via_pjrt(nc, in_maps, n_cores=N_CORES), None, None

    tmpdir = tempfile.mkdtemp(prefix="bass_ntff_")
    try:
        with hook(tmpdir, [0]):
            results = bass2jax.run_bass_via_pjrt(nc, in_maps, n_cores=N_CORES)
    except Exception as e:
        print(f"[kernel] NTFF hook failed ({type(e).__name__}: {e}); plain run")
        return bass2jax.run_bass_via_pjrt(nc, in_maps, n_cores=N_CORES), None, None

    exec_ns = None
    try:
        if glob.glob(os.path.join(tmpdir, "*_body*.ntff")):
            import gauge.profiler
            from concourse._compat import FishPath

            profile_obj = gauge.profiler.Profile(
                profile_path=FishPath(tmpdir),
                kernel_dev_mode=True,
                profile_on_exit=False,
                bass_kernel=nc.m,
                offline_processing=True,
                fname="*_body*",
            )
            prs = profile_obj.to_perfetto(model_index=(0,))
            if prs:
                exec_ns = max(p.exec_time_ns for p in prs if p.exec_time_ns)
    except Exception as e:
        print(f"[kernel] NTFF parse failed ({type(e).__name__}: {e})")
    return results, exec_ns, tmpdir


def kernel(**inputs):
    global LAST_EXEC_NS, LAST_TRACE_DIR
    if "nc" not in _CACHE:
        nc = _build_nc()
        if not nc.is_finalized():
            nc.finalize()
        _CACHE["nc"] = nc
    nc = _CACHE["nc"]

    in_maps = _host_prep(**inputs)
    results, exec_ns, trace_dir = _run(nc, in_maps)
    LAST_EXEC_NS = exec_ns
    LAST_TRACE_DIR = trace_dir
    out = np.concatenate(
        [np.ascontiguousarray(results[c]["out"].T) for c in range(N_CORES)], axis=0
    )
    return out.astype(np.float32)


# revision 12
# speedup vs baseline: 1.0325x; 1.0002x over previous
"""Trainium2 Bass kernel for the Actor MLP (BatchNorm -> 3-layer MLP -> atan2).

Data-parallel across 8 NeuronCores: batch sharded 8192 rows/core, weights
replicated. BatchNorm batch stats via per-shard bn_stats + 8KB AllReduce.

v2 design (from baseline trace analysis; baseline HW 2.10-2.20ms, this
kernel measured 1.941-1.948ms, rel err bit-identical 1.980e-2):
- x is pre-transposed on the host into [16, 128, 8, 512] tiles, removing
  all 1024 PE transposes (~131us) and the PSUM/ACT eviction pressure;
  bn_stats runs directly on the SBUF tiles with the exact same [128, 512]
  shapes/order as before (bit-identical stats -> identical rounding, since
  the 1.98e-2 rel err sits ~6 sign flips under the 2e-2 gate).
- The atan2 result stays in [action, batch] layout on device; the host
  transposes it back. L3 accumulates ty/tx in PSUM interleaved with L2
  (one m-slice behind) so h2 never fully materializes in SBUF. Batch
  tiles are processed in PAIRS sharing each streamed weight slice, which
  halves weight HBM traffic (403MB -> 202MB/core).
- Phase 2 runs with ZERO PE gaps >1.5us: 6656 matmuls at the f32r floor.

Known hardware floor (measured, don't re-litigate): each f32r matmul is
preceded by its own 218ns LDWEIGHTS (fp32-mode loads get no background
weight buffer and walrus re-emits LDW even for an identical stationary,
so pairing does NOT amortize it) -> issue cadence is ~262ns/MM, i.e.
6656 MMs = 1.75ms of the runtime. Failed experiments: x/out DMAs on the
ACT HWDGE ring (head-of-line blocks ACTIVATEs, +56us), phase-1 x loads
via gpsimd SWDGE (+32us), quarter-tile phase-1 DMAs (+70us),
reciprocal_approx_fast in the epilogue (NaN on device: tx can reach
denormal/zero even though the CPU-simulated min |tx| is 6.5e-9).

Matmuls run in float32r (fp32 with the mantissa rounded to ~11 bits).
All matmul operand values, accumulation orders, and the bn_stats/bn_aggr
shapes are bit-identical to the validated baseline; only the epilogue
(sign corrections are exact) could tolerate precision changes.
"""

import numpy as np

P = 128
B_CORE = 8192            # batch rows per core
BT = 512                 # batch tile (free dim of transposed activations)
NBT = B_CORE // BT       # 16
NPAIR = NBT // 2         # 8 pairs of batch tiles sharing weight loads
D_IN = 1024
K_IN = D_IN // P         # 8
D_H = 2048
K_H = D_H // P           # 16
D_ACT = 128
BN_EPS = 1e-5
N_CORES = 8
INV_PI = float(1.0 / np.pi)

_CACHE = {}

LAST_EXEC_NS = None
LAST_TRACE_DIR = None


def _build_nc():
    import concourse.mybir as mybir
    import concourse.tile as tile
    from concourse import bacc

    f32 = mybir.dt.float32
    f32r = mybir.dt.float32r
    AF = mybir.ActivationFunctionType
    ALU = mybir.AluOpType

    nc = bacc.Bacc()

    # x pre-transposed on host: [t, p, k, b] = x[t*512 + b, k*128 + p]
    x_ext = nc.declare_dram_parameter("x", [NBT, P, K_IN, BT], f32, isOutput=False)
    # weights pre-tiled on host: [m_slice, partition, k*128] so each m-slice
    # DMA reads one fully-contiguous block per partition
    w1t_ext = nc.declare_dram_parameter("w1t", [K_H, P, D_IN], f32r, isOutput=False)
    w2t_ext = nc.declare_dram_parameter("w2t", [K_H, P, D_H], f32r, isOutput=False)
    w3yt_ext = nc.declare_dram_parameter("w3yt", [D_H, D_ACT], f32r, isOutput=False)
    w3xt_ext = nc.declare_dram_parameter("w3xt", [D_H, D_ACT], f32r, isOutput=False)
    b1_ext = nc.declare_dram_parameter("b1r", [P, K_H], f32, isOutput=False)
    b2_ext = nc.declare_dram_parameter("b2r", [P, K_H], f32, isOutput=False)
    b3y_ext = nc.declare_dram_parameter("b3y", [P, 1], f32, isOutput=False)
    b3x_ext = nc.declare_dram_parameter("b3x", [P, 1], f32, isOutput=False)
    bnw_ext = nc.declare_dram_parameter("bnw", [P, K_IN], f32, isOutput=False)
    bnb_ext = nc.declare_dram_parameter("bnb", [P, K_IN], f32, isOutput=False)
    # output in transposed [action, batch] layout; host transposes back
    out_ext = nc.declare_dram_parameter("out", [D_ACT, B_CORE], f32, isOutput=True)

    with tile.TileContext(nc) as tc:
        with (
            tc.tile_pool(name="singles", bufs=1) as singles,
            tc.tile_pool(name="small", bufs=1) as small,
            tc.tile_pool(name="raw", bufs=4) as raw_pool,
            tc.tile_pool(name="xn", bufs=1) as xt_pool,
            tc.tile_pool(name="w1m", bufs=3) as w1_pool,
            tc.tile_pool(name="w2m", bufs=2) as w2_pool,
            tc.tile_pool(name="hid", bufs=1) as hid_pool,
            tc.tile_pool(name="h2s", bufs=4) as h2s_pool,
            tc.tile_pool(name="epi", bufs=1) as epi_pool,
            tc.tile_pool(name="mmpsum", bufs=4, space="PSUM") as mm_pool,
            tc.tile_pool(name="l3psum", bufs=1, space="PSUM") as l3_pool,
            tc.tile_pool(name="dram", bufs=1, space="DRAM") as dram_pool,
        ):
            # ---- constants ----
            bnws = singles.tile([P, K_IN], f32)
            nc.sync.dma_start(out=bnws, in_=bnw_ext[:])
            bnbs = singles.tile([P, K_IN], f32)
            nc.sync.dma_start(out=bnbs, in_=bnb_ext[:])

            # ---- pass 1: batch stats straight off the host-transposed tiles
            # (same [128, 512] per-(k,t) bn_stats and per-k bn_aggr as the
            # validated baseline -> bit-identical mean/var) ----
            stats = small.tile([P, K_IN, NBT, 6], f32)
            mv = small.tile([P, K_IN, 2], f32)
            # pk = [mean, E[x^2]] / N_CORES, packed for the AllReduce
            pk = small.tile([P, K_IN, 2], f32)
            KH2 = K_IN // 2
            for t in range(NBT):
                for h in range(2):
                    raw = raw_pool.tile([P, KH2, BT], f32, tag="raw", name="rawh")
                    nc.sync.dma_start(out=raw, in_=x_ext[t, :, h * KH2 : (h + 1) * KH2])
                    for kk in range(KH2):
                        k = h * KH2 + kk
                        nc.vector.bn_stats(out=stats[:, k, t, :], in_=raw[:, kk, :])
                        if t == NBT - 1:
                            # aggregate per k as soon as its last bn_stats lands
                            nc.vector.bn_aggr(out=mv[:, k, :], in_=stats[:, k, :, :])

            # pass-2 constants, loaded behind the stats pass
            w3ys = singles.tile([P, K_H, D_ACT], f32r)
            nc.sync.dma_start(out=w3ys, in_=w3yt_ext[:].rearrange("(k p) a -> p k a", p=P))
            w3xs = singles.tile([P, K_H, D_ACT], f32r)
            nc.sync.dma_start(out=w3xs, in_=w3xt_ext[:].rearrange("(k p) a -> p k a", p=P))
            b1s = singles.tile([P, K_H], f32)
            nc.sync.dma_start(out=b1s, in_=b1_ext[:])
            b2s = singles.tile([P, K_H], f32)
            nc.sync.dma_start(out=b2s, in_=b2_ext[:])
            b3ys = singles.tile([P, 1], f32)
            nc.sync.dma_start(out=b3ys, in_=b3y_ext[:])
            b3xs = singles.tile([P, 1], f32)
            nc.sync.dma_start(out=b3xs, in_=b3x_ext[:])

            # pack [mean, E[x^2]]/N_CORES for the AllReduce
            nc.vector.tensor_scalar_mul(pk[:, :, 0], mv[:, :, 0], 1.0 / N_CORES)
            nc.vector.tensor_mul(pk[:, :, 1], mv[:, :, 0], mv[:, :, 0])
            nc.vector.tensor_add(pk[:, :, 1], pk[:, :, 1], mv[:, :, 1])
            nc.vector.tensor_scalar_mul(pk[:, :, 1], pk[:, :, 1], 1.0 / N_CORES)

            cc_in = dram_pool.tile([P, K_IN, 2], f32)
            cc_out = dram_pool.tile([P, K_IN, 2], f32)
            # gpsimd's queue, so this 8KB doesn't sit behind weight MBs
            nc.gpsimd.dma_start(out=cc_in, in_=pk)
            nc.gpsimd.collective_compute(
                "AllReduce",
                ALU.add,
                replica_groups=[list(range(N_CORES))],
                ins=[cc_in.opt()],
                outs=[cc_out.opt()],
            )
            g = small.tile([P, K_IN, 2], f32)
            nc.gpsimd.dma_start(out=g, in_=cc_out)

            # global mean / var -> per-feature scale & shift
            gm = g[:, :, 0]
            var = small.tile([P, K_IN], f32)
            nc.vector.tensor_copy(out=var, in_=g[:, :, 1])
            gm2 = small.tile([P, K_IN], f32)
            nc.vector.tensor_mul(gm2, gm, gm)
            nc.vector.tensor_sub(var, var, gm2)
            eps_t = small.tile([P, 1], f32)
            nc.vector.memset(eps_t, BN_EPS)
            sq = small.tile([P, K_IN], f32)
            nc.scalar.activation(out=sq, in_=var, func=AF.Sqrt, bias=eps_t, scale=1.0)
            rstd = small.tile([P, K_IN], f32)
            nc.vector.reciprocal(out=rstd, in_=sq)
            scale = small.tile([P, K_IN], f32)
            nc.vector.tensor_mul(scale, bnws, rstd)
            shift = small.tile([P, K_IN], f32)
            nc.vector.tensor_mul(shift, gm, scale)
            nc.vector.tensor_sub(shift, bnbs, shift)

            # ---- pass 2: normalize + 3-layer MLP + atan2, paired batch tiles
            # so each stationary weight tile feeds two N=512 matmuls ----
            for pr in range(NPAIR):
                tiles = (2 * pr, 2 * pr + 1)
                xts = []
                for t in tiles:
                    xt = xt_pool.tile([P, K_IN, BT], f32r, tag=f"xt{t % 2}")
                    for h in range(2):
                        raw = raw_pool.tile([P, KH2, BT], f32, tag="raw", name="rawh")
                        nc.sync.dma_start(
                            out=raw, in_=x_ext[t, :, h * KH2 : (h + 1) * KH2]
                        )
                        for kk in range(KH2):
                            k = h * KH2 + kk
                            nc.scalar.activation(
                                out=xt[:, k, :],
                                in_=raw[:, kk, :],
                                func=AF.Identity,
                                bias=shift[:, k : k + 1],
                                scale=scale[:, k : k + 1],
                            )
                    xts.append(xt)

                # layer 1: h1T = relu(W1 @ xnormT + b1), W1 streamed by m-slice
                h1s = [
                    hid_pool.tile([P, K_H, BT], f32r, tag=f"h1{i}", name=f"h1{i}")
                    for i in range(2)
                ]
                for m in range(K_H):
                    w1m = w1_pool.tile([P, K_IN, P], f32r, tag="w1m")
                    nc.sync.dma_start(
                        out=w1m,
                        in_=w1t_ext[m].rearrange("p (k c) -> p k c", k=K_IN),
                    )
                    accs = [
                        mm_pool.tile([P, BT], f32, tag="mm", name=f"acc{i}")
                        for i in range(2)
                    ]
                    for k in range(K_IN):
                        for i in range(2):
                            nc.tensor.matmul(
                                accs[i],
                                w1m[:, k, :],
                                xts[i][:, k, :],
                                start=(k == 0),
                                stop=(k == K_IN - 1),
                            )
                    for i in range(2):
                        nc.scalar.activation(
                            out=h1s[i][:, m, :],
                            in_=accs[i],
                            func=AF.Relu,
                            bias=b1s[:, m : m + 1],
                            scale=1.0,
                        )

                # layer 2 + layer 3 interleaved: as each h2 m-slice is evicted,
                # its ty/tx contributions accumulate one m-iteration later so
                # the PE never waits on the ACT eviction.
                tys = [
                    l3_pool.tile([P, BT], f32, tag=f"ty{i}", name=f"ty{i}")
                    for i in range(2)
                ]
                txs = [
                    l3_pool.tile([P, BT], f32, tag=f"tx{i}", name=f"tx{i}")
                    for i in range(2)
                ]
                h2hist = {}
                for m in range(K_H):
                    w2m = w2_pool.tile([P, K_H, P], f32r, tag="w2m")
                    nc.sync.dma_start(
                        out=w2m,
                        in_=w2t_ext[m].rearrange("p (k c) -> p k c", k=K_H),
                    )
                    accs = [
                        mm_pool.tile([P, BT], f32, tag="mm", name=f"acc{i}")
                        for i in range(2)
                    ]
                    for k in range(K_H):
                        for i in range(2):
                            nc.tensor.matmul(
                                accs[i],
                                w2m[:, k, :],
                                h1s[i][:, k, :],
                                start=(k == 0),
                                stop=(k == K_H - 1),
                            )
                    h2m = [
                        h2s_pool.tile([P, BT], f32r, tag="h2s", name=f"h2s{i}")
                        for i in range(2)
                    ]
                    for i in range(2):
                        nc.scalar.activation(
                            out=h2m[i],
                            in_=accs[i],
                            func=AF.Relu,
                            bias=b2s[:, m : m + 1],
                            scale=1.0,
                        )
                    h2hist[m] = h2m
                    if m >= 1:
                        _emit_l3(nc, m - 1, h2hist.pop(m - 1), w3ys, w3xs, tys, txs, K_H)
                _emit_l3(nc, K_H - 1, h2hist.pop(K_H - 1), w3ys, w3xs, tys, txs, K_H)

                # tanh eviction + atan2 epilogue per tile (off the PE)
                for i, t in enumerate(tiles):
                    ty = epi_pool.tile([P, BT], f32, tag="ty")
                    tx = epi_pool.tile([P, BT], f32, tag="tx")
                    nc.scalar.activation(out=ty, in_=tys[i], func=AF.Tanh, bias=b3ys, scale=1.0)
                    nc.scalar.activation(out=tx, in_=txs[i], func=AF.Tanh, bias=b3xs, scale=1.0)

                    # atan2(ty, tx)/pi = Arctan(ty/tx)/pi + sign(ty)*(1-sign(tx))/2
                    rx = epi_pool.tile([P, BT], f32, tag="rx")
                    nc.vector.reciprocal(out=rx, in_=tx)
                    nc.vector.tensor_mul(rx, ty, rx)            # q = ty/tx
                    sy = epi_pool.tile([P, BT], f32, tag="sy")
                    nc.scalar.activation(out=sy, in_=ty, func=AF.Sign)
                    sx = epi_pool.tile([P, BT], f32, tag="sx")
                    nc.scalar.activation(out=sx, in_=tx, func=AF.Sign)
                    nc.scalar.activation(out=tx, in_=rx, func=AF.Arctan)  # a (tx dead)
                    nc.vector.tensor_mul(sx, sy, sx)            # sy*sx
                    nc.vector.tensor_sub(sy, sy, sx)            # d = sy*(1-sx)
                    nc.vector.tensor_scalar(
                        out=rx, in0=tx, scalar1=INV_PI, scalar2=None, op0=ALU.mult
                    )
                    nc.vector.tensor_scalar(
                        out=sy, in0=sy, scalar1=0.5, scalar2=None, op0=ALU.mult
                    )
                    nc.vector.tensor_add(rx, rx, sy)            # resT
                    nc.sync.dma_start(
                        out=out_ext[:, t * BT : (t + 1) * BT], in_=rx
                    )

    return nc


def _emit_l3(nc, m, h2m, w3ys, w3xs, tys, txs, K_H):
    """ty/tx += w3{y,x}[m] @ h2[m] for both tiles of the pair; each stationary
    serves two matmuls. PSUM accumulation order over m matches the baseline."""
    for ws, dsts in ((w3ys, tys), (w3xs, txs)):
        for i in range(2):
            nc.tensor.matmul(
                dsts[i],
                ws[:, m, :],
                h2m[i],
                start=(m == 0),
                stop=(m == K_H - 1),
            )


def _round_f32r(a):
    """Round-to-nearest to f32r granularity (low 12 mantissa bits zeroed)."""
    a = np.ascontiguousarray(np.asarray(a, np.float32))
    b = a.view(np.uint32)
    b = ((b + 0x800) & np.uint32(0xFFFFF000)).astype(np.uint32)
    return b.view(np.float32)


def _tile_w(w, k_tiles):
    """[M, K] row-major -> [M/128, 128p, K] where [m, p, k*128+c] = w[m*128+c, k*128+p]."""
    m_tiles = w.shape[0] // P
    t = w.reshape(m_tiles, P, k_tiles, P).transpose(0, 3, 2, 1)
    return np.ascontiguousarray(t.reshape(m_tiles, P, k_tiles * P))


def _host_prep(states, bn_weight, bn_bias, w1, b1, w2, b2, w3, b3):
    w1t = _round_f32r(_tile_w(np.asarray(w1, np.float32), K_IN))
    w2t = _round_f32r(_tile_w(np.asarray(w2, np.float32), K_H))
    w3 = np.asarray(w3, np.float32)
    w3yt = _round_f32r(w3[0::2].T)   # [D_H, D_ACT]
    w3xt = _round_f32r(w3[1::2].T)
    b1r = np.ascontiguousarray(np.asarray(b1, np.float32).reshape(K_H, P).T)
    b2r = np.ascontiguousarray(np.asarray(b2, np.float32).reshape(K_H, P).T)
    b3 = np.asarray(b3, np.float32)
    b3y = np.ascontiguousarray(b3[0::2].reshape(P, 1))
    b3x = np.ascontiguousarray(b3[1::2].reshape(P, 1))
    bnw = np.ascontiguousarray(np.asarray(bn_weight, np.float32).reshape(K_IN, P).T)
    bnb = np.ascontiguousarray(np.asarray(bn_bias, np.float32).reshape(K_IN, P).T)
    shared = {
        "w1t": w1t, "w2t": w2t, "w3yt": w3yt, "w3xt": w3xt,
        "b1r": b1r, "b2r": b2r, "b3y": b3y, "b3x": b3x,
        "bnw": bnw, "bnb": bnb,
    }
    states = np.asarray(states, np.float32)
    in_maps = []
    for c in range(N_CORES):
        m = dict(shared)
        shard = states[c * B_CORE : (c + 1) * B_CORE]
        # [t, p, k, b] = x[t*512 + b, k*128 + p]
        m["x"] = np.ascontiguousarray(
            shard.reshape(NBT, BT, K_IN, P).transpose(0, 3, 2, 1)
        )
        in_maps.append(m)
    return in_maps


def _get_ntff_hook():
    """Best-effort NTFF profiling hook (axon images without antenv.axon_hooks)."""
    try:
        from antenv.axon_hooks import get_axon_ntff_profile_hook

        return get_axon_ntff_profile_hook()
    except ImportError:
        pass
    try:
        from trn_agent_boot.trn_boot import _ntff_profile_via_ctypes

        return _ntff_profile_via_ctypes("/opt/axon/libaxon_pjrt.so")
    except Exception:
        return None


def _run(nc, in_maps, profile=True):
    """Run the SPMD kernel via PJRT; return (per-core results, exec_time_ns)."""
    import glob
    import os
    import tempfile

    from concourse import bass2jax

    hook = _get_ntff_hook() if profile else None
    if hook is None:
        return bass2jax.run_bass_via_pjrt(nc, in_maps, n_cores=N_CORES), None, None

    tmpdir = tempfile.mkdtemp(prefix="bass_ntff_")
    try:
        with hook(tmpdir, [0]):
            results = bass2jax.run_bass_via_pjrt(nc, in_maps, n_cores=N_CORES)
    except Exception as e:
        print(f"[kernel] NTFF hook failed ({type(e).__name__}: {e}); plain run")
        return bass2jax.run_bass_via_pjrt(nc, in_maps, n_cores=N_CORES), None, None

    exec_ns = None
    try:
        if glob.glob(os.path.join(tmpdir, "*_body*.ntff")):
            import gauge.profiler
            from concourse._compat import FishPath

            profile_obj = gauge.profiler.Profile(
                profile_path=FishPath(tmpdir),
                kernel_dev_mode=True,
                profile_on_exit=False,
                bass_kernel=nc.m,
                offline_processing=True,
                fname="*_body*",
            )
            prs = profile_obj.to_perfetto(model_index=(0,))
            if prs:
                exec_ns = max(p.exec_time_ns for p in prs if p.exec_time_ns)
    except Exception as e:
        print(f"[kernel] NTFF parse failed ({type(e).__name__}: {e})")
    return results, exec_ns, tmpdir


def kernel(**inputs):
    global LAST_EXEC_NS, LAST_TRACE_DIR
    if "nc" not in _CACHE:
        nc = _build_nc()
        if not nc.is_finalized():
            nc.finalize()
        _CACHE["nc"] = nc
    nc = _CACHE["nc"]

    in_maps = _host_prep(**inputs)
    results, exec_ns, trace_dir = _run(nc, in_maps)
    LAST_EXEC_NS = exec_ns
    LAST_TRACE_DIR = trace_dir
    out = np.concatenate(
        [np.ascontiguousarray(results[c]["out"].T) for c in range(N_CORES)], axis=0
    )
    return out.astype(np.float32)
